# revision 21
# baseline (speedup 1.0000x reference)
"""Mixed-score multi-head attention Trainium2 kernel.

Sharding: 8 cores = 4 batches x 2 head-quads. Each core computes, for its
batch b and its 4 heads, the full attention and a PARTIAL output projection
(its heads' slice of the recombine matmul). Host sums the two partials per
batch.

Per-core layout (H4 = 4 local heads, q = 512, k = 512):
- hidden pre-relu tiles [(s4, k32) = 128 partitions, q = 512] per (head, B, sc)
  built by two row-packed matmuls into PSUM:
    dot:    lhsT = K[32d @ base 32j, 32 k-cols].bcast(s4)   (K = 32)
    affine: lhsT = bpat (b_s/a_s delta pattern)             (K = 32, cost rows)
- relu evac PSUM->SBUF, one op per tile:
    ACT tiles: relu(a*x + c)            (scale/bias per-partition APs)
    DVE tiles: max(sign(a)*x, -c/|a|)   (tensor_scalar mult/max, per-part APs)
  mix2 weights per tile form: ACT: w ; DVE: w*|a| (constant folds out of
  softmax since it is uniform over k within a head).
- mix2: col-packed [K=128, M=32] matmuls -> scores^T [(4h,32k), q] PSUM
- exp (no max subtraction; |scores| < 3) -> E in SBUF
- AV: per head [K=32, M=32] matmuls with replicated V, accumulated over B
- sumexp via [K=128, M=4] head-sum pattern matmul, accumulated over B
- Zrecip -> broadcast matmul -> normalize att during evac -> output proj.
"""

import os
import sys
import numpy as np

import concourse.bacc as bacc
import concourse.mybir as mybir
import concourse.tile as tile
from concourse.bass_utils import run_bass_kernel_spmd


def _install_ntff_hook():
    """Provide antenv.axon_hooks (absent in this image) so trace=True can
    capture NTFF profiles via the injected libaxon_pjrt.so C ABI."""
    if "antenv.axon_hooks" in sys.modules:
        return
    import types
    import ctypes
    import contextlib

    so_path = "/opt/axon/libaxon_pjrt.so"
    hook = None
    if os.path.exists(so_path):
        lib = ctypes.CDLL(so_path)
        if hasattr(lib, "axon_start_nrt_profile"):
            lib.axon_start_nrt_profile.argtypes = [
                ctypes.POINTER(ctypes.c_int64), ctypes.c_size_t]
            lib.axon_start_nrt_profile.restype = ctypes.c_int64
            lib.axon_stop_nrt_profile.argtypes = [ctypes.c_char_p]
            lib.axon_stop_nrt_profile.restype = ctypes.c_int64

            @contextlib.contextmanager
            def _hook(output_dir, device_ids):
                import jax
                jax.devices()
                if device_ids:
                    ids = (ctypes.c_int64 * len(device_ids))(*device_ids)
                    rc = lib.axon_start_nrt_profile(ids, len(device_ids))
                else:
                    rc = lib.axon_start_nrt_profile(None, 0)
                if rc != 0:
                    raise RuntimeError(f"axon_start_nrt_profile rc={rc}")
                try:
                    yield
                finally:
                    n = lib.axon_stop_nrt_profile(str(output_dir).encode())
                    print(f"profile: {n} file(s) written to {output_dir}",
                          file=sys.stderr)
            hook = _hook
    mod = types.ModuleType("antenv.axon_hooks")
    mod.get_axon_ntff_profile_hook = lambda: hook
    mod.set_axon_ntff_profile_hook = lambda h: None
    sys.modules["antenv.axon_hooks"] = mod

f32 = mybir.dt.float32
bf16 = mybir.dt.bfloat16
MM_FAST = os.environ.get("MSK_MM_DT", "bf16") == "bf16"
fmm = bf16 if MM_FAST else f32
AF = mybir.ActivationFunctionType
ALU = mybir.AluOpType

B_, L, D, H, DK, MS = 4, 512, 256, 8, 32, 16
NB = 16          # number of 32-wide k blocks
NSC = 4          # number of s-chunks (4 s values each)
# engine assignment for relu evac: per row-group j (0..3): True -> ACT
# waves: A = (j0 scalar, j1 vector), B = (j2 scalar, j3 vector) so each
# engine drains one bank per wave and bank WAR deps pipeline across steps.
ACT_J = (True, False, True, False)

_compiled = {}
_last_results = None


# --------------------------------------------------------------------------
# device program
# --------------------------------------------------------------------------
def build_program():
    nc = bacc.Bacc("TRN2", target_bir_lowering=False, debug=False)

    # consolidated input blobs: one f32 blob, one bf16 blob, cost in 4 chunks
    # blob32 cols: qT 1024 | wk 512 | wq 512 | zpat 128 | cvec 16 = 2192
    blob32 = nc.dram_tensor("blob32", [128, 2192], f32, kind="ExternalInput").ap()
    # blob16 cols: wv 256 | wo 256 | bpat 512 | wpat 512 | spat 4 | ak 512 = 2052
    blob16 = nc.dram_tensor("blob16", [128, 2052], fmm, kind="ExternalInput").ap()
    costp = nc.dram_tensor("costp", [NB, 128, 512], fmm, kind="ExternalInput").ap()     # cost[b].T rows 32B..32B+32 replicated 4x
    out_d = nc.dram_tensor("out", [512, 256], f32, kind="ExternalOutput").ap()

    with tile.TileContext(nc) as tc:
        _build(nc, tc, blob32, blob16, costp, out_d)
    nc.compile()
    return nc


def _build(nc, tc, blob32, blob16, costp, out_d):
    import contextlib
    ctx = contextlib.ExitStack()
    sb = ctx.enter_context
    # ---- static SBUF ----
    b32_sb = sb(nc.sbuf_tensor([128, 2192], f32))
    b16_sb = sb(nc.sbuf_tensor([128, 2052], fmm))
    cost_sb = sb(nc.sbuf_tensor([128, NB * 512], fmm))
    qT_sb = b32_sb[:, 0:1024]                  # D-chunk c at cols 512c
    wk_sb = b32_sb[:, 1024:1536]
    wq_sb = b32_sb[:, 1536:2048]
    zpat_sb = b32_sb[:, 2048:2176]
    cvec_sb = b32_sb[:, 2176:2192]             # col sc*4+j: c (ACT) / -c (DVE)
    wv_sb = b16_sb[:, 0:256]
    wo_sb = b16_sb[:, 256:512]
    bpat_sb = b16_sb[:, 512:1024]              # b-diag, rows 32j, per sc
    wpat_sb = b16_sb[:, 1024:1536]
    spat_sb = b16_sb[:, 1536:1540]
    ak_sb = b16_sb[:, 1540:2052]               # a[h(p), s] bcast over k32
    K_sb = sb(nc.sbuf_tensor([128, 512], fmm))            # [(4h,32d), k]
    Q_sb = sb(nc.sbuf_tensor([128, 512], fmm))            # [(4h,32d), q]
    Vr_sb = sb(nc.sbuf_tensor([128, NB * 128], fmm))      # [(4rep,32k), (h,d)] per B
    K_bc4 = [sb(nc.sbuf_tensor(f"K_bc4_{i}", [128, NB * 128], fmm))
             for i in range(NSC)]              # a*K [(4h,32d),(B,s4,k32)] per sc
    qTb = sb(nc.sbuf_tensor([128, 2 * NB * 128], fmm))    # [(D), (c,B,rep4,k32)]
    hid_sb = sb(nc.sbuf_tensor([128, 3 * 4 * 512], fmm))  # 3 rounds x 4 tiles
    E_sb = sb(nc.sbuf_tensor([128, 3 * 512], fmm))        # 3 B-slots
    zr_sb = sb(nc.sbuf_tensor([128, 512], f32))           # rows 0-3 used
    zb_sb = sb(nc.sbuf_tensor([128, 512], f32))
    att_sb = sb(nc.sbuf_tensor([128, 512], fmm))
    out_sb = sb(nc.sbuf_tensor([128, 4 * 256], f32))
    # ---- PSUM (8 banks) ----
    hid_ps = [sb(nc.psum_tensor(f"hid_ps{i}", [128, 512], f32))
              for i in range(4)]
    sc_ps = [sb(nc.psum_tensor(f"sc_ps{i}", [128, 512], f32))
             for i in range(2)]
    att_ps = sb(nc.psum_tensor("att_ps", [128, 512], f32))
    sum_ps = sb(nc.psum_tensor("sum_ps", [128, 512], f32))

    def hid_tile(j):
        return hid_ps[j][:]

    dma = nc.sync.dma_start
    # ---- loads (few big DMAs; cost in 4 chunks for early loop start) ----
    dma(b32_sb[:], blob32[:, :])
    dma(b16_sb[:], blob16[:, :])
    for g in range(4):
        dma(cost_sb[:, 2048 * g:2048 * (g + 1)]
            .rearrange("p (b k) -> p b k", b=4),
            costp[4 * g:4 * (g + 1)].rearrange("b p k -> p b k"))

    mm = nc.tensor.matmul

    # ---- K / Q projections: out [(4h,32d), n] ----
    # lhsT = hsel chunk [128, 256->quad cols?]: hsel[c] = Wk-like selector...
    # We instead compute full-H projection then keep quad cols via hsel trick:
    # simpler: lhsT = wk chunk cols (host already sliced to this quad's 128).
    # wk/wq hold the FULL 256 cols; host supplies hsel as the quad's 128 col
    # one-hot selector so the same program works for both quads.
    # K = (hsel.T @ wk).T ... to keep it simple we do two matmuls:
    #   tmp[(hq,d), k] = sum_c wkq_c.T @ qT_c  with wkq = wk @ hsel (host-side)
    # -> host bakes the quad slice directly into wk/wq/wv/wo; hsel unused.
    for c in range(2):
        mm(hid_ps[0][:], wk_sb[:, 256 * c:256 * c + 128], qT_sb[:, 512 * c:512 * (c + 1)],
           start=(c == 0), stop=(c == 1), tile_position=(0, 0))
    nc.vector.tensor_copy(K_sb[:], hid_ps[0][:])
    for c in range(2):
        mm(hid_ps[1][:], wq_sb[:, 256 * c:256 * c + 128], qT_sb[:, 512 * c:512 * (c + 1)],
           start=(c == 0), stop=(c == 1), tile_position=(0, 0))
    nc.vector.tensor_copy(Q_sb[:], hid_ps[1][:])

    # ---- K_bc4 = a_s * K, s4-broadcast, per-sc variants ----
    for sc in range(NSC):
        nc.vector.tensor_tensor(
            K_bc4[sc][:].rearrange("p (b s k) -> p b s k", s=4, k=32),
            K_sb[:].rearrange("p (b k) -> p b k", k=32)
                .unsqueeze(2).broadcast_to((128, NB, 4, 32)),
            ak_sb[:, 128 * sc:128 * (sc + 1)]
                .rearrange("p (s k) -> p s k", k=32)
                .unsqueeze(1).broadcast_to((128, NB, 4, 32)),
            op=ALU.mult)
    for c in range(2):
        nc.vector.tensor_copy(
            qTb[:, 2048 * c:2048 * (c + 1)]
                .rearrange("p (b s k) -> p b s k", s=4, k=32),
            qT_sb[:, 512 * c:512 * (c + 1)]
                .rearrange("p (b k) -> p b k", k=32)
                .unsqueeze(2).broadcast_to((128, NB, 4, 32)))

    # ---- V replicated: Vr[B] [(4rep,32k), (h,d)] ----
    for g in range(4):           # 4 banks x 4 B each
        for i in range(4):
            Bb = 4 * g + i
            for c in range(2):
                lhsT = qTb[:, 2048 * c + 128 * Bb: 2048 * c + 128 * (Bb + 1)]
                mm(hid_ps[g][:, 128 * i:128 * (i + 1)], lhsT,
                   wv_sb[:, 128 * c:128 * (c + 1)],
                   start=(c == 0), stop=(c == 1), tile_position=(0, 0))
        if g % 2 == 0:
            nc.scalar.copy(Vr_sb[:, 512 * g:512 * (g + 1)], hid_ps[g][:])
        else:
            nc.vector.tensor_copy(Vr_sb[:, 512 * g:512 * (g + 1)], hid_ps[g][:])

    # ---- main loop ----
    # hidden pre-act y = a*dot + b*cost + c built fully in PSUM:
    #   dot: lhsT = K_bc4 (a-scaled K, s4-bcast), K=32, 4-row-packed
    #   affine: K=64 (cost rows + ones rows), 2+2 packed; adds b*cost + c
    # evacs are param-free -> merged 2-bank [128,1024] ops, scalar|vector.
    def emit_round(Bb, sc):
        slot = (Bb * NSC + sc) % 3
        hbase = 2048 * slot
        for wave in (0,):
            js = (0, 1, 2, 3)
            for j in js:
                lhsT = K_bc4[sc][32 * j:32 * j + 32, 128 * Bb:128 * (Bb + 1)]
                mm(hid_tile(j), lhsT, Q_sb[32 * j:32 * j + 32, :],
                   start=True, stop=False, tile_position=(32 * j, 0))
            for j in js:
                mm(hid_tile(j),
                   bpat_sb[32 * j:32 * j + 32, 128 * sc:128 * (sc + 1)],
                   cost_sb[32 * j:32 * j + 32, 512 * Bb:512 * (Bb + 1)],
                   start=False, stop=True, tile_position=(32 * j, 0))
            for j in js:
                t = sc * 4 + j
                dst = hid_sb[:, hbase + 512 * j: hbase + 512 * (j + 1)]
                if j % 2 == 0:
                    nc.scalar.activation(dst, hid_tile(j), AF.Relu,
                                         bias=cvec_sb[:, t:t + 1])
                else:
                    nc.vector.tensor_scalar(dst, hid_tile(j),
                                            cvec_sb[:, t:t + 1], None,
                                            op0=ALU.max)

    def emit_mix2(Bb, sc):
        slot = (Bb * NSC + sc) % 3
        hbase = 2048 * slot
        sps = sc_ps[Bb % 2]
        for j in range(4):
            mm(sps[32 * j:32 * j + 32, :],
               wpat_sb[:, 128 * sc + 32 * j: 128 * sc + 32 * (j + 1)],
               hid_sb[:, hbase + 512 * j: hbase + 512 * (j + 1)],
               start=(sc == 0), stop=(sc == NSC - 1), tile_position=(0, 32 * j),
               skip_group_check=True)

    def emit_exp(Bb):
        nc.scalar.activation(E_sb[:, 512 * (Bb % 3):512 * (Bb % 3 + 1)],
                             sc_ps[Bb % 2][:], AF.Exp)

    def emit_av(Bb):
        for j in range(4):
            mm(att_ps[32 * j:32 * j + 32, :],
               Vr_sb[32 * j:32 * j + 32, 128 * Bb + 32 * j:128 * Bb + 32 * (j + 1)],
               E_sb[32 * j:32 * j + 32, 512 * (Bb % 3):512 * (Bb % 3 + 1)],
               start=(Bb == 0), stop=(Bb == NB - 1), tile_position=(32 * j, 32 * j),
               skip_group_check=True)
        mm(sum_ps[0:4, :], spat_sb[:],
           E_sb[:, 512 * (Bb % 3):512 * (Bb % 3 + 1)],
           start=(Bb == 0), stop=(Bb == NB - 1), tile_position=(0, 0),
           skip_group_check=True)

    # software pipeline: mix2 lags rounds by one step; exp after mix2(sc=3);
    # AV lags exp by one B.
    steps = [(Bb, sc) for Bb in range(NB) for sc in range(NSC)]
    for idx, (Bb, sc) in enumerate(steps):
        emit_round(Bb, sc)
        if idx >= 1:
            pB, psc = steps[idx - 1]
            emit_mix2(pB, psc)
            if psc == NSC - 1:
                emit_exp(pB)
                if pB >= 1:
                    emit_av(pB - 1)
    emit_mix2(*steps[-1])
    emit_exp(NB - 1)
    emit_av(NB - 2)
    emit_av(NB - 1)

    # ---- tail: normalize + output projection ----
    nc.vector.reciprocal_approx_fast(zr_sb[0:4, :], sum_ps[0:4, :])
    mm(sc_ps[0][:], zpat_sb[0:4, 0:128], zr_sb[0:4, :],
       start=True, stop=True, tile_position=(0, 0))
    nc.scalar.copy(zb_sb[:], sc_ps[0][:])
    nc.vector.tensor_tensor(att_sb[:], att_ps[:], zb_sb[:], op=ALU.mult)
    for qc in range(4):
        ps = sc_ps[qc % 2]
        half = 256 * (qc // 2)
        mm(ps[:, half:half + 256], att_sb[:, 128 * qc:128 * (qc + 1)],
           wo_sb[:], start=True, stop=True, tile_position=(0, 0))
        eng = nc.vector.tensor_copy if qc % 2 else nc.scalar.copy
        eng(out_sb[:, 256 * qc:256 * (qc + 1)], ps[:, half:half + 256])
    dma(out_d.rearrange("(g p) d -> p g d", g=4),
        out_sb[:].rearrange("p (g d) -> p g d", g=4))
    ctx.close()


# --------------------------------------------------------------------------
# host-side input prep
# --------------------------------------------------------------------------
def make_core_inputs(inputs, core):
    b, quad = core // 2, core % 2
    queries = inputs["queries"][b]            # [512, 256]
    cost = inputs["cost_mat"][b]              # [512, 512]
    a = inputs["mix1_w"][:, 0, :]             # [H, MS]
    bb = inputs["mix1_w"][:, 1, :]
    cc = inputs["mix1_b"]                     # [H, MS]
    w2 = inputs["mix2_w"][:, :, 0]            # [H, MS]
    hs = slice(quad * 4 * DK, (quad + 1) * 4 * DK)

    qT = np.ascontiguousarray(queries.T).reshape(2, 128, 512)
    costT = np.ascontiguousarray(cost.T)      # [k, q]
    # cost replicas: rows 0-31 / 64-95 = cost block, rows 32-63 / 96-127 = ones
    # (the ones rows feed the affine matmul's c-bias contraction)
    costp = np.empty((NB, 128, 512), np.float32)
    for Bb in range(NB):
        blk = costT[32 * Bb:32 * Bb + 32, :]
        costp[Bb] = np.tile(blk, (4, 1))
    wk = np.ascontiguousarray(inputs["Wk"]).reshape(2, 128, 256)
    wq = (np.ascontiguousarray(inputs["Wq"]) * (DK ** -0.5)).astype(np.float32).reshape(2, 128, 256)
    # K/Q proj in the program use cols [256c : 256c+128] -> must be the quad's
    # 128 cols: bake quad slice so lhsT slice [*, :128] is the quad cols.
    wk = np.ascontiguousarray(wk[:, :, hs])   # [2,128,128]
    wq = np.ascontiguousarray(wq[:, :, hs])
    wk = np.concatenate([wk, np.zeros_like(wk)], axis=2)  # pad back to 256 cols
    wq = np.concatenate([wq, np.zeros_like(wq)], axis=2)
    wv = np.ascontiguousarray(inputs["Wv"][:, hs]).reshape(2, 128, 128)
    wo = np.ascontiguousarray(inputs["Wo"][hs, :])        # [128, 256]

    # bpat: b-diag at rows 32j (K=32, cost rhs); cvec: +c for ACT tiles
    # (j even, relu(x+c)), -c for DVE tiles (j odd, max(x,-c); the dropped
    # +c*w2 sum is k-uniform per head -> softmax-invariant).
    bpat = np.zeros((128, NSC * 128), np.float32)
    cvec = np.zeros((128, 16), np.float32)
    wpat = np.zeros((128, NSC * 128), np.float32)
    ak = np.zeros((128, 512), np.float32)
    rows = np.arange(32)
    for sc in range(NSC):
        for j in range(4):
            h = quad * 4 + j
            sgn = 1.0 if j % 2 == 0 else -1.0
            for si in range(4):
                s = sc * 4 + si
                bpat[32 * j + rows, 128 * sc + 32 * si + rows] = bb[h, s]
                cvec[32 * si + rows, sc * 4 + j] = sgn * cc[h, s]
                wpat[32 * si + rows, 128 * sc + 32 * j + rows] = w2[h, s]
    for p in range(128):
        h = quad * 4 + p // 32
        for s in range(MS):
            ak[p, 32 * s:32 * (s + 1)] = a[h, s]
    spat = np.zeros((128, 4), np.float32)
    for j in range(4):
        spat[32 * j:32 * (j + 1), j] = 1.0
    zpat = np.zeros((128, 128), np.float32)
    for j in range(4):
        zpat[j, 32 * j:32 * (j + 1)] = 1.0
    import ml_dtypes
    mmdt = ml_dtypes.bfloat16 if MM_FAST else np.float32
    # blob32 cols: qT 1024 | wk 512 | wq 512 | zpat 128 | cvec 16
    blob32 = np.concatenate(
        [qT[0], qT[1], wk[0][:, :256], wk[1][:, :256],
         wq[0][:, :256], wq[1][:, :256], zpat, cvec],
        axis=1).astype(np.float32)
    # blob16 cols: wv 256 | wo 256 | bpat 512 | wpat 512 | spat 4 | ak 512
    blob16 = np.concatenate(
        [wv[0], wv[1], wo, bpat, wpat, spat, ak],
        axis=1).astype(mmdt)
    return dict(blob32=np.ascontiguousarray(blob32),
                blob16=np.ascontiguousarray(blob16),
                costp=costp.astype(mmdt))


def kernel(**inputs):
    global _last_results
    inputs = {k: np.asarray(v, np.float32) for k, v in inputs.items()}
    if "nc" not in _compiled:
        _compiled["nc"] = build_program()
    nc = _compiled["nc"]
    in_maps = [make_core_inputs(inputs, core) for core in range(8)]
    trace = bool(os.environ.get("MSK_TRACE"))
    if trace:
        _install_ntff_hook()
    res = run_bass_kernel_spmd(nc, in_maps, list(range(8)), trace=trace)
    _last_results = res
    out = np.zeros((B_, L, D), np.float32)
    for core in range(8):
        out[core // 2] += res.results[core]["out"]
    return out



# revision 22
# speedup vs baseline: 1.3182x; 1.3182x over previous
"""Mixed-score multi-head attention Trainium2 kernel.

Sharding: 8 cores = 4 batches x 2 head-quads. Each core computes, for its
batch b and its 4 heads, the full attention and a PARTIAL output projection
(its heads' slice of the recombine matmul). Host sums the two partials per
batch.

Per-core layout (H4 = 4 local heads, q = 512, k = 512):
- hidden pre-relu tiles [(s4, k32) = 128 partitions, q = 512] per (head, B, sc)
  built by two row-packed matmuls into PSUM:
    dot:    lhsT = K[32d @ base 32j, 32 k-cols].bcast(s4)   (K = 32)
    affine: lhsT = bpat (b_s/a_s delta pattern)             (K = 32, cost rows)
- relu evac PSUM->SBUF, one op per tile:
    ACT tiles: relu(a*x + c)            (scale/bias per-partition APs)
    DVE tiles: max(sign(a)*x, -c/|a|)   (tensor_scalar mult/max, per-part APs)
  mix2 weights per tile form: ACT: w ; DVE: w*|a| (constant folds out of
  softmax since it is uniform over k within a head).
- mix2: col-packed [K=128, M=32] matmuls -> scores^T [(4h,32k), q] PSUM
- exp (no max subtraction; |scores| < 3) -> E in SBUF
- AV: per head [K=32, M=32] matmuls with replicated V, accumulated over B
- sumexp via [K=128, M=4] head-sum pattern matmul, accumulated over B
- Zrecip -> broadcast matmul -> normalize att during evac -> output proj.
"""

import os
import sys
import numpy as np

import concourse.bacc as bacc
import concourse.mybir as mybir
import concourse.tile as tile
from concourse.bass_utils import run_bass_kernel_spmd


def _install_ntff_hook():
    """Provide antenv.axon_hooks (absent in this image) so trace=True can
    capture NTFF profiles via the injected libaxon_pjrt.so C ABI."""
    if "antenv.axon_hooks" in sys.modules:
        return
    import types
    import ctypes
    import contextlib

    so_path = "/opt/axon/libaxon_pjrt.so"
    hook = None
    if os.path.exists(so_path):
        lib = ctypes.CDLL(so_path)
        if hasattr(lib, "axon_start_nrt_profile"):
            lib.axon_start_nrt_profile.argtypes = [
                ctypes.POINTER(ctypes.c_int64), ctypes.c_size_t]
            lib.axon_start_nrt_profile.restype = ctypes.c_int64
            lib.axon_stop_nrt_profile.argtypes = [ctypes.c_char_p]
            lib.axon_stop_nrt_profile.restype = ctypes.c_int64

            @contextlib.contextmanager
            def _hook(output_dir, device_ids):
                import jax
                jax.devices()
                if device_ids:
                    ids = (ctypes.c_int64 * len(device_ids))(*device_ids)
                    rc = lib.axon_start_nrt_profile(ids, len(device_ids))
                else:
                    rc = lib.axon_start_nrt_profile(None, 0)
                if rc != 0:
                    raise RuntimeError(f"axon_start_nrt_profile rc={rc}")
                try:
                    yield
                finally:
                    n = lib.axon_stop_nrt_profile(str(output_dir).encode())
                    print(f"profile: {n} file(s) written to {output_dir}",
                          file=sys.stderr)
            hook = _hook
    mod = types.ModuleType("antenv.axon_hooks")
    mod.get_axon_ntff_profile_hook = lambda: hook
    mod.set_axon_ntff_profile_hook = lambda h: None
    sys.modules["antenv.axon_hooks"] = mod

f32 = mybir.dt.float32
bf16 = mybir.dt.bfloat16
MM_FAST = os.environ.get("MSK_MM_DT", "bf16") == "bf16"
fmm = bf16 if MM_FAST else f32
AF = mybir.ActivationFunctionType
ALU = mybir.AluOpType

B_, L, D, H, DK, MS = 4, 512, 256, 8, 32, 16
NB = 16          # number of 32-wide k blocks
NSC = 4          # number of s-chunks (4 s values each)
# engine assignment for relu evac: per row-group j (0..3): True -> ACT
# waves: A = (j0 scalar, j1 vector), B = (j2 scalar, j3 vector) so each
# engine drains one bank per wave and bank WAR deps pipeline across steps.
ACT_J = (True, False, True, False)

_compiled = {}
_last_results = None


# --------------------------------------------------------------------------
# device program
# --------------------------------------------------------------------------
def build_program():
    nc = bacc.Bacc("TRN2", target_bir_lowering=False, debug=False)

    # consolidated input blobs: one f32 blob, one bf16 blob, cost in 4 chunks
    # blob32 cols: zpat 128 | cvec 16 = 144
    blob32 = nc.dram_tensor("blob32", [128, 144], f32, kind="ExternalInput").ap()
    # blob16 cols: qT 1024 | wk 512 | wq 512 | wv 256 | wo 256 | bpat 512 |
    #              wpat 512 | spat 4 | ak 512 = 4100
    blob16 = nc.dram_tensor("blob16", [128, 4100], fmm, kind="ExternalInput").ap()
    costp = nc.dram_tensor("costp", [NB, 128, 512], fmm, kind="ExternalInput").ap()     # cost[b].T rows 32B..32B+32 replicated 4x
    out_d = nc.dram_tensor("out", [512, 256], f32, kind="ExternalOutput").ap()

    with tile.TileContext(nc) as tc:
        _build(nc, tc, blob32, blob16, costp, out_d)
    nc.compile()
    return nc


def _build(nc, tc, blob32, blob16, costp, out_d):
    import contextlib
    ctx = contextlib.ExitStack()
    sb = ctx.enter_context
    # ---- static SBUF ----
    b32_sb = sb(nc.sbuf_tensor([128, 144], f32))
    b16_sb = sb(nc.sbuf_tensor([128, 4100], fmm))
    cost_sb = sb(nc.sbuf_tensor([128, NB * 512], fmm))
    zpat_sb = b32_sb[:, 0:128]
    cvec_sb = b32_sb[:, 128:144]               # col sc*4+j: c (ACT) / -c (DVE)
    qT_sb = b16_sb[:, 0:1024]                  # D-chunk c at cols 512c
    wk_sb = b16_sb[:, 1024:1536]
    wq_sb = b16_sb[:, 1536:2048]
    wv_sb = b16_sb[:, 2048:2304]
    wo_sb = b16_sb[:, 2304:2560]
    bpat_sb = b16_sb[:, 2560:3072]             # b-diag, rows 32j, per sc
    wpat_sb = b16_sb[:, 3072:3584]
    spat_sb = b16_sb[:, 3584:3588]
    ak_sb = b16_sb[:, 3588:4100]               # a[h(p), s] bcast over k32
    K_sb = sb(nc.sbuf_tensor([128, 512], fmm))            # [(4h,32d), k]
    Q_sb = sb(nc.sbuf_tensor([128, 512], fmm))            # [(4h,32d), q]
    Vr_sb = sb(nc.sbuf_tensor([128, NB * 128], fmm))      # [(4rep,32k), (h,d)] per B
    K_bc4 = [sb(nc.sbuf_tensor(f"K_bc4_{i}", [128, NB * 128], fmm))
             for i in range(NSC)]              # a*K [(4h,32d),(B,s4,k32)] per sc
    qTb = sb(nc.sbuf_tensor([128, 2 * NB * 128], fmm))    # [(D), (c,B,rep4,k32)]
    hid_sb = sb(nc.sbuf_tensor([128, 3 * 4 * 512], fmm))  # 3 rounds x 4 tiles
    E_sb = sb(nc.sbuf_tensor([128, 3 * 512], fmm))        # 3 B-slots
    zr_sb = sb(nc.sbuf_tensor([128, 512], f32))           # rows 0-3 used
    zb_sb = sb(nc.sbuf_tensor([128, 512], f32))
    att_sb = sb(nc.sbuf_tensor([128, 512], fmm))
    out_sb = sb(nc.sbuf_tensor([128, 4 * 256], f32))
    # ---- PSUM (8 banks) ----
    hid_ps = [sb(nc.psum_tensor(f"hid_ps{i}", [128, 512], f32))
              for i in range(4)]
    sc_ps = [sb(nc.psum_tensor(f"sc_ps{i}", [128, 512], f32))
             for i in range(2)]
    att_ps = sb(nc.psum_tensor("att_ps", [128, 512], f32))
    sum_ps = sb(nc.psum_tensor("sum_ps", [128, 512], f32))

    def hid_tile(j):
        return hid_ps[j][:]

    dma = nc.sync.dma_start
    # ---- loads (few big DMAs; cost in 4 chunks for early loop start) ----
    dma(b32_sb[:], blob32[:, :])
    dma(b16_sb[:], blob16[:, :])
    for g in range(4):
        dma(cost_sb[:, 2048 * g:2048 * (g + 1)]
            .rearrange("p (b k) -> p b k", b=4),
            costp[4 * g:4 * (g + 1)].rearrange("b p k -> p b k"))

    mm = nc.tensor.matmul

    # ---- K / Q projections: out [(4h,32d), n] ----
    # lhsT = hsel chunk [128, 256->quad cols?]: hsel[c] = Wk-like selector...
    # We instead compute full-H projection then keep quad cols via hsel trick:
    # simpler: lhsT = wk chunk cols (host already sliced to this quad's 128).
    # wk/wq hold the FULL 256 cols; host supplies hsel as the quad's 128 col
    # one-hot selector so the same program works for both quads.
    # K = (hsel.T @ wk).T ... to keep it simple we do two matmuls:
    #   tmp[(hq,d), k] = sum_c wkq_c.T @ qT_c  with wkq = wk @ hsel (host-side)
    # -> host bakes the quad slice directly into wk/wq/wv/wo; hsel unused.
    for c in range(2):
        mm(hid_ps[0][:], wk_sb[:, 256 * c:256 * c + 128], qT_sb[:, 512 * c:512 * (c + 1)],
           start=(c == 0), stop=(c == 1), tile_position=(0, 0))
    nc.vector.tensor_copy(K_sb[:], hid_ps[0][:])
    for c in range(2):
        mm(hid_ps[1][:], wq_sb[:, 256 * c:256 * c + 128], qT_sb[:, 512 * c:512 * (c + 1)],
           start=(c == 0), stop=(c == 1), tile_position=(0, 0))
    nc.vector.tensor_copy(Q_sb[:], hid_ps[1][:])

    # ---- K_bc4 = a_s * K, s4-broadcast, per-sc variants ----
    for sc in range(NSC):
        nc.vector.tensor_tensor(
            K_bc4[sc][:].rearrange("p (b s k) -> p b s k", s=4, k=32),
            K_sb[:].rearrange("p (b k) -> p b k", k=32)
                .unsqueeze(2).broadcast_to((128, NB, 4, 32)),
            ak_sb[:, 128 * sc:128 * (sc + 1)]
                .rearrange("p (s k) -> p s k", k=32)
                .unsqueeze(1).broadcast_to((128, NB, 4, 32)),
            op=ALU.mult)
    for c in range(2):
        nc.vector.tensor_copy(
            qTb[:, 2048 * c:2048 * (c + 1)]
                .rearrange("p (b s k) -> p b s k", s=4, k=32),
            qT_sb[:, 512 * c:512 * (c + 1)]
                .rearrange("p (b k) -> p b k", k=32)
                .unsqueeze(2).broadcast_to((128, NB, 4, 32)))

    # ---- V replicated: Vr[B] [(4rep,32k), (h,d)] ----
    for g in range(4):           # 4 banks x 4 B each
        for i in range(4):
            Bb = 4 * g + i
            for c in range(2):
                lhsT = qTb[:, 2048 * c + 128 * Bb: 2048 * c + 128 * (Bb + 1)]
                mm(hid_ps[g][:, 128 * i:128 * (i + 1)], lhsT,
                   wv_sb[:, 128 * c:128 * (c + 1)],
                   start=(c == 0), stop=(c == 1), tile_position=(0, 0))
        if g % 2 == 0:
            nc.scalar.copy(Vr_sb[:, 512 * g:512 * (g + 1)], hid_ps[g][:])
        else:
            nc.vector.tensor_copy(Vr_sb[:, 512 * g:512 * (g + 1)], hid_ps[g][:])

    # ---- main loop ----
    # hidden pre-act y = a*dot + b*cost + c built fully in PSUM:
    #   dot: lhsT = K_bc4 (a-scaled K, s4-bcast), K=32, 4-row-packed
    #   affine: K=64 (cost rows + ones rows), 2+2 packed; adds b*cost + c
    # evacs are param-free -> merged 2-bank [128,1024] ops, scalar|vector.
    def emit_round(Bb, sc):
        slot = (Bb * NSC + sc) % 3
        hbase = 2048 * slot
        for wave in (0, 1):
            js = (0, 1) if wave == 0 else (2, 3)
            for j in js:
                lhsT = K_bc4[sc][32 * j:32 * j + 32, 128 * Bb:128 * (Bb + 1)]
                mm(hid_tile(j), lhsT, Q_sb[32 * j:32 * j + 32, :],
                   start=True, stop=False, tile_position=(32 * j, 0))
            for j in js:
                mm(hid_tile(j),
                   bpat_sb[32 * j:32 * j + 32, 128 * sc:128 * (sc + 1)],
                   cost_sb[32 * j:32 * j + 32, 512 * Bb:512 * (Bb + 1)],
                   start=False, stop=True, tile_position=(32 * j, 0))
            for j in js:
                t = sc * 4 + j
                dst = hid_sb[:, hbase + 512 * j: hbase + 512 * (j + 1)]
                if j % 2 == 0:
                    nc.scalar.activation(dst, hid_tile(j), AF.Relu,
                                         bias=cvec_sb[:, t:t + 1])
                else:
                    nc.vector.tensor_scalar(dst, hid_tile(j),
                                            cvec_sb[:, t:t + 1], None,
                                            op0=ALU.max)

    def emit_mix2(Bb, sc):
        slot = (Bb * NSC + sc) % 3
        hbase = 2048 * slot
        sps = sc_ps[Bb % 2]
        for j in range(4):
            mm(sps[32 * j:32 * j + 32, :],
               wpat_sb[:, 128 * sc + 32 * j: 128 * sc + 32 * (j + 1)],
               hid_sb[:, hbase + 512 * j: hbase + 512 * (j + 1)],
               start=(sc == 0), stop=(sc == NSC - 1), tile_position=(0, 32 * j),
               skip_group_check=True)

    def emit_exp(Bb):
        nc.scalar.activation(E_sb[:, 512 * (Bb % 3):512 * (Bb % 3 + 1)],
                             sc_ps[Bb % 2][:], AF.Exp)

    def emit_av(Bb):
        for j in range(4):
            mm(att_ps[32 * j:32 * j + 32, :],
               Vr_sb[32 * j:32 * j + 32, 128 * Bb + 32 * j:128 * Bb + 32 * (j + 1)],
               E_sb[32 * j:32 * j + 32, 512 * (Bb % 3):512 * (Bb % 3 + 1)],
               start=(Bb == 0), stop=(Bb == NB - 1), tile_position=(32 * j, 32 * j),
               skip_group_check=True)
        mm(sum_ps[0:4, :], spat_sb[:],
           E_sb[:, 512 * (Bb % 3):512 * (Bb % 3 + 1)],
           start=(Bb == 0), stop=(Bb == NB - 1), tile_position=(0, 0),
           skip_group_check=True)

    # software pipeline: mix2 lags rounds by one step; exp after mix2(sc=3);
    # AV lags exp by one B.
    steps = [(Bb, sc) for Bb in range(NB) for sc in range(NSC)]
    for idx, (Bb, sc) in enumerate(steps):
        emit_round(Bb, sc)
        if idx >= 1:
            pB, psc = steps[idx - 1]
            emit_mix2(pB, psc)
            if psc == NSC - 1:
                emit_exp(pB)
                if pB >= 1:
                    emit_av(pB - 1)
    emit_mix2(*steps[-1])
    emit_exp(NB - 1)
    emit_av(NB - 2)
    emit_av(NB - 1)

    # ---- tail: normalize + output projection ----
    nc.vector.reciprocal_approx_fast(zr_sb[0:4, :], sum_ps[0:4, :])
    mm(sc_ps[0][:], zpat_sb[0:4, 0:128], zr_sb[0:4, :],
       start=True, stop=True, tile_position=(0, 0))
    nc.scalar.copy(zb_sb[:], sc_ps[0][:])
    nc.vector.tensor_tensor(att_sb[:], att_ps[:], zb_sb[:], op=ALU.mult)
    for qc in range(4):
        ps = sc_ps[qc % 2]
        half = 256 * (qc // 2)
        mm(ps[:, half:half + 256], att_sb[:, 128 * qc:128 * (qc + 1)],
           wo_sb[:], start=True, stop=True, tile_position=(0, 0))
        eng = nc.vector.tensor_copy if qc % 2 else nc.scalar.copy
        eng(out_sb[:, 256 * qc:256 * (qc + 1)], ps[:, half:half + 256])
    dma(out_d.rearrange("(g p) d -> p g d", g=4),
        out_sb[:].rearrange("p (g d) -> p g d", g=4))
    ctx.close()


# --------------------------------------------------------------------------
# host-side input prep
# --------------------------------------------------------------------------
def make_core_inputs(inputs, core):
    b, quad = core // 2, core % 2
    queries = inputs["queries"][b]            # [512, 256]
    cost = inputs["cost_mat"][b]              # [512, 512]
    a = inputs["mix1_w"][:, 0, :]             # [H, MS]
    bb = inputs["mix1_w"][:, 1, :]
    cc = inputs["mix1_b"]                     # [H, MS]
    w2 = inputs["mix2_w"][:, :, 0]            # [H, MS]
    hs = slice(quad * 4 * DK, (quad + 1) * 4 * DK)

    qT = np.ascontiguousarray(queries.T).reshape(2, 128, 512)
    costT = np.ascontiguousarray(cost.T)      # [k, q]
    # cost replicas: rows 0-31 / 64-95 = cost block, rows 32-63 / 96-127 = ones
    # (the ones rows feed the affine matmul's c-bias contraction)
    costp = np.empty((NB, 128, 512), np.float32)
    for Bb in range(NB):
        blk = costT[32 * Bb:32 * Bb + 32, :]
        costp[Bb] = np.tile(blk, (4, 1))
    wk = np.ascontiguousarray(inputs["Wk"]).reshape(2, 128, 256)
    wq = (np.ascontiguousarray(inputs["Wq"]) * (DK ** -0.5)).astype(np.float32).reshape(2, 128, 256)
    # K/Q proj in the program use cols [256c : 256c+128] -> must be the quad's
    # 128 cols: bake quad slice so lhsT slice [*, :128] is the quad cols.
    wk = np.ascontiguousarray(wk[:, :, hs])   # [2,128,128]
    wq = np.ascontiguousarray(wq[:, :, hs])
    wk = np.concatenate([wk, np.zeros_like(wk)], axis=2)  # pad back to 256 cols
    wq = np.concatenate([wq, np.zeros_like(wq)], axis=2)
    wv = np.ascontiguousarray(inputs["Wv"][:, hs]).reshape(2, 128, 128)
    wo = np.ascontiguousarray(inputs["Wo"][hs, :])        # [128, 256]

    # bpat: b-diag at rows 32j (K=32, cost rhs); cvec: +c for ACT tiles
    # (j even, relu(x+c)), -c for DVE tiles (j odd, max(x,-c); the dropped
    # +c*w2 sum is k-uniform per head -> softmax-invariant).
    bpat = np.zeros((128, NSC * 128), np.float32)
    cvec = np.zeros((128, 16), np.float32)
    wpat = np.zeros((128, NSC * 128), np.float32)
    ak = np.zeros((128, 512), np.float32)
    rows = np.arange(32)
    for sc in range(NSC):
        for j in range(4):
            h = quad * 4 + j
            sgn = 1.0 if j % 2 == 0 else -1.0
            for si in range(4):
                s = sc * 4 + si
                bpat[32 * j + rows, 128 * sc + 32 * si + rows] = bb[h, s]
                cvec[32 * si + rows, sc * 4 + j] = sgn * cc[h, s]
                wpat[32 * si + rows, 128 * sc + 32 * j + rows] = w2[h, s]
    for p in range(128):
        h = quad * 4 + p // 32
        for s in range(MS):
            ak[p, 32 * s:32 * (s + 1)] = a[h, s]
    spat = np.zeros((128, 4), np.float32)
    for j in range(4):
        spat[32 * j:32 * (j + 1), j] = 1.0
    zpat = np.zeros((128, 128), np.float32)
    for j in range(4):
        zpat[j, 32 * j:32 * (j + 1)] = 1.0
    import ml_dtypes
    mmdt = ml_dtypes.bfloat16 if MM_FAST else np.float32
    # blob32 cols: zpat 128 | cvec 16
    blob32 = np.concatenate([zpat, cvec], axis=1).astype(np.float32)
    # blob16 cols: qT 1024 | wk 512 | wq 512 | wv 256 | wo 256 | bpat 512 |
    #              wpat 512 | spat 4 | ak 512
    blob16 = np.concatenate(
        [qT[0], qT[1], wk[0][:, :256], wk[1][:, :256],
         wq[0][:, :256], wq[1][:, :256], wv[0], wv[1], wo,
         bpat, wpat, spat, ak],
        axis=1).astype(mmdt)
    return dict(blob32=np.ascontiguousarray(blob32),
                blob16=np.ascontiguousarray(blob16),
                costp=costp.astype(mmdt))


def kernel(**inputs):
    global _last_results
    inputs = {k: np.asarray(v, np.float32) for k, v in inputs.items()}
    if "nc" not in _compiled:
        _compiled["nc"] = build_program()
    nc = _compiled["nc"]
    in_maps = [make_core_inputs(inputs, core) for core in range(8)]
    trace = bool(os.environ.get("MSK_TRACE"))
    if trace:
        _install_ntff_hook()
    res = run_bass_kernel_spmd(nc, in_maps, list(range(8)), trace=trace)
    _last_results = res
    out = np.zeros((B_, L, D), np.float32)
    for core in range(8):
        out[core // 2] += res.results[core]["out"]
    return out



# revision 23
# speedup vs baseline: 1.4865x; 1.1277x over previous
"""Mixed-score multi-head attention Trainium2 kernel.

Sharding: 8 cores = 4 batches x 2 head-quads. Each core computes, for its
batch b and its 4 heads, the full attention and a PARTIAL output projection
(its heads' slice of the recombine matmul). Host sums the two partials per
batch.

Per-core layout (H4 = 4 local heads, q = 512, k = 512):
- hidden pre-relu tiles [(s4, k32) = 128 partitions, q = 512] per (head, B, sc)
  built by two row-packed matmuls into PSUM:
    dot:    lhsT = K[32d @ base 32j, 32 k-cols].bcast(s4)   (K = 32)
    affine: lhsT = bpat (b_s/a_s delta pattern)             (K = 32, cost rows)
- relu evac PSUM->SBUF, one op per tile:
    ACT tiles: relu(a*x + c)            (scale/bias per-partition APs)
    DVE tiles: max(sign(a)*x, -c/|a|)   (tensor_scalar mult/max, per-part APs)
  mix2 weights per tile form: ACT: w ; DVE: w*|a| (constant folds out of
  softmax since it is uniform over k within a head).
- mix2: col-packed [K=128, M=32] matmuls -> scores^T [(4h,32k), q] PSUM
- exp (no max subtraction; |scores| < 3) -> E in SBUF
- AV: per head [K=32, M=32] matmuls with replicated V, accumulated over B
- sumexp via [K=128, M=4] head-sum pattern matmul, accumulated over B
- Zrecip -> broadcast matmul -> normalize att during evac -> output proj.
"""

import os
import sys
import numpy as np

import concourse.bacc as bacc
import concourse.mybir as mybir
import concourse.tile as tile
from concourse.bass_utils import run_bass_kernel_spmd


def _install_ntff_hook():
    """Provide antenv.axon_hooks (absent in this image) so trace=True can
    capture NTFF profiles via the injected libaxon_pjrt.so C ABI."""
    if "antenv.axon_hooks" in sys.modules:
        return
    import types
    import ctypes
    import contextlib

    so_path = "/opt/axon/libaxon_pjrt.so"
    hook = None
    if os.path.exists(so_path):
        lib = ctypes.CDLL(so_path)
        if hasattr(lib, "axon_start_nrt_profile"):
            lib.axon_start_nrt_profile.argtypes = [
                ctypes.POINTER(ctypes.c_int64), ctypes.c_size_t]
            lib.axon_start_nrt_profile.restype = ctypes.c_int64
            lib.axon_stop_nrt_profile.argtypes = [ctypes.c_char_p]
            lib.axon_stop_nrt_profile.restype = ctypes.c_int64

            @contextlib.contextmanager
            def _hook(output_dir, device_ids):
                import jax
                jax.devices()
                if device_ids:
                    ids = (ctypes.c_int64 * len(device_ids))(*device_ids)
                    rc = lib.axon_start_nrt_profile(ids, len(device_ids))
                else:
                    rc = lib.axon_start_nrt_profile(None, 0)
                if rc != 0:
                    raise RuntimeError(f"axon_start_nrt_profile rc={rc}")
                try:
                    yield
                finally:
                    n = lib.axon_stop_nrt_profile(str(output_dir).encode())
                    print(f"profile: {n} file(s) written to {output_dir}",
                          file=sys.stderr)
            hook = _hook
    mod = types.ModuleType("antenv.axon_hooks")
    mod.get_axon_ntff_profile_hook = lambda: hook
    mod.set_axon_ntff_profile_hook = lambda h: None
    sys.modules["antenv.axon_hooks"] = mod

f32 = mybir.dt.float32
bf16 = mybir.dt.bfloat16
MM_FAST = os.environ.get("MSK_MM_DT", "bf16") == "bf16"
fmm = bf16 if MM_FAST else f32
AF = mybir.ActivationFunctionType
ALU = mybir.AluOpType

B_, L, D, H, DK, MS = 4, 512, 256, 8, 32, 16
NB = 16          # number of 32-wide k blocks
NSC = 4          # number of s-chunks (4 s values each)
# engine assignment for relu evac: per row-group j (0..3): True -> ACT
# waves: A = (j0 scalar, j1 vector), B = (j2 scalar, j3 vector) so each
# engine drains one bank per wave and bank WAR deps pipeline across steps.
ACT_J = (True, False, True, False)

_compiled = {}
_last_results = None


# --------------------------------------------------------------------------
# device program
# --------------------------------------------------------------------------
def build_program():
    nc = bacc.Bacc("TRN2", target_bir_lowering=False, debug=False)

    # consolidated input blobs: one f32 blob, one bf16 blob, cost in 4 chunks
    # blob32 cols: zpat 128 | cvec 16 = 144
    blob32 = nc.dram_tensor("blob32", [128, 144], f32, kind="ExternalInput").ap()
    # blob16 cols: qT 1024 | wk 512 | wq 512 | wv 256 | wo 256 | bpat 512 |
    #              wpat 512 | spat 4 | ak 512 = 4100
    blob16 = nc.dram_tensor("blob16", [128, 4100], fmm, kind="ExternalInput").ap()
    costp = nc.dram_tensor("costp", [NB, 128, 512], fmm, kind="ExternalInput").ap()     # cost[b].T rows 32B..32B+32 replicated 4x
    out_d = nc.dram_tensor("out", [512, 256], f32, kind="ExternalOutput").ap()

    with tile.TileContext(nc) as tc:
        _build(nc, tc, blob32, blob16, costp, out_d)
    nc.compile()
    return nc


def _build(nc, tc, blob32, blob16, costp, out_d):
    import contextlib
    ctx = contextlib.ExitStack()
    sb = ctx.enter_context
    # ---- static SBUF ----
    b32_sb = sb(nc.sbuf_tensor([128, 144], f32))
    b16_sb = sb(nc.sbuf_tensor([128, 4100], fmm))
    cost_sb = sb(nc.sbuf_tensor([128, NB * 512], fmm))
    zpat_sb = b32_sb[:, 0:128]
    cvec_sb = b32_sb[:, 128:144]               # col sc*4+j: c (ACT) / -c (DVE)
    qT_sb = b16_sb[:, 0:1024]                  # D-chunk c at cols 512c
    wk_sb = b16_sb[:, 1024:1536]
    wq_sb = b16_sb[:, 1536:2048]
    wv_sb = b16_sb[:, 2048:2304]
    wo_sb = b16_sb[:, 2304:2560]
    bpat_sb = b16_sb[:, 2560:3072]             # b-diag, rows 32j, per sc
    wpat_sb = b16_sb[:, 3072:3584]
    spat_sb = b16_sb[:, 3584:3588]
    ak_sb = b16_sb[:, 3588:4100]               # a[h(p), s] bcast over k32
    K_sb = sb(nc.sbuf_tensor([128, 512], fmm))            # [(4h,32d), k]
    Q_sb = sb(nc.sbuf_tensor([128, 512], fmm))            # [(4h,32d), q]
    Vr_sb = sb(nc.sbuf_tensor([128, NB * 128], fmm))      # [(4rep,32k), (h,d)] per B
    K_bc4 = [sb(nc.sbuf_tensor(f"K_bc4_{i}", [128, NB * 128], fmm))
             for i in range(NSC)]              # a*K [(4h,32d),(B,s4,k32)] per sc
    qTb = sb(nc.sbuf_tensor([128, 2 * NB * 128], fmm))    # [(D), (c,B,rep4,k32)]
    hid_sb = sb(nc.sbuf_tensor([128, 3 * 4 * 512], fmm))  # 3 rounds x 4 tiles
    E_sb = sb(nc.sbuf_tensor([128, 3 * 512], fmm))        # 3 B-slots
    zr_sb = sb(nc.sbuf_tensor([128, 512], f32))           # rows 0-3 used
    zb_sb = sb(nc.sbuf_tensor([128, 512], f32))
    att_sb = sb(nc.sbuf_tensor([128, 512], fmm))
    out_sb = sb(nc.sbuf_tensor([128, 4 * 256], f32))
    # ---- PSUM (8 banks) ----
    hid_ps = [sb(nc.psum_tensor(f"hid_ps{i}", [128, 512], f32))
              for i in range(4)]
    sc_ps = [sb(nc.psum_tensor(f"sc_ps{i}", [128, 512], f32))
             for i in range(2)]
    att_ps = sb(nc.psum_tensor("att_ps", [128, 512], f32))
    sum_ps = sb(nc.psum_tensor("sum_ps", [128, 512], f32))

    def hid_tile(j):
        return hid_ps[j][:]

    dma = nc.sync.dma_start
    # ---- loads (few big DMAs; cost in 4 chunks for early loop start) ----
    dma(b32_sb[:], blob32[:, :])
    dma(b16_sb[:], blob16[:, :])
    for g in range(4):
        dma(cost_sb[:, 2048 * g:2048 * (g + 1)]
            .rearrange("p (b k) -> p b k", b=4),
            costp[4 * g:4 * (g + 1)].rearrange("b p k -> p b k"))

    mm = nc.tensor.matmul

    # ---- K / Q projections: out [(4h,32d), n] ----
    # lhsT = hsel chunk [128, 256->quad cols?]: hsel[c] = Wk-like selector...
    # We instead compute full-H projection then keep quad cols via hsel trick:
    # simpler: lhsT = wk chunk cols (host already sliced to this quad's 128).
    # wk/wq hold the FULL 256 cols; host supplies hsel as the quad's 128 col
    # one-hot selector so the same program works for both quads.
    # K = (hsel.T @ wk).T ... to keep it simple we do two matmuls:
    #   tmp[(hq,d), k] = sum_c wkq_c.T @ qT_c  with wkq = wk @ hsel (host-side)
    # -> host bakes the quad slice directly into wk/wq/wv/wo; hsel unused.
    for c in range(2):
        mm(hid_ps[0][:], wk_sb[:, 256 * c:256 * c + 128], qT_sb[:, 512 * c:512 * (c + 1)],
           start=(c == 0), stop=(c == 1), tile_position=(0, 0))
    nc.vector.tensor_copy(K_sb[:], hid_ps[0][:])
    for c in range(2):
        mm(hid_ps[1][:], wq_sb[:, 256 * c:256 * c + 128], qT_sb[:, 512 * c:512 * (c + 1)],
           start=(c == 0), stop=(c == 1), tile_position=(0, 0))
    nc.vector.tensor_copy(Q_sb[:], hid_ps[1][:])

    # ---- K_bc4 = a_s * K, s4-broadcast, per-sc variants ----
    for sc in range(NSC):
        nc.gpsimd.tensor_tensor(
            K_bc4[sc][:].rearrange("p (b s k) -> p b s k", s=4, k=32),
            K_sb[:].rearrange("p (b k) -> p b k", k=32)
                .unsqueeze(2).broadcast_to((128, NB, 4, 32)),
            ak_sb[:, 128 * sc:128 * (sc + 1)]
                .rearrange("p (s k) -> p s k", k=32)
                .unsqueeze(1).broadcast_to((128, NB, 4, 32)),
            op=ALU.mult)
    for c in range(2):
        nc.vector.tensor_copy(
            qTb[:, 2048 * c:2048 * (c + 1)]
                .rearrange("p (b s k) -> p b s k", s=4, k=32),
            qT_sb[:, 512 * c:512 * (c + 1)]
                .rearrange("p (b k) -> p b k", k=32)
                .unsqueeze(2).broadcast_to((128, NB, 4, 32)))

    # ---- V replicated: Vr[B] [(4rep,32k), (h,d)]; scratch in sc/att/sum
    # banks so loop step 0 never waits on the hid banks. ----
    vscr = [sc_ps[0], sc_ps[1], att_ps, sum_ps]
    for g in range(4):           # 4 banks x 4 B each
        for i in range(4):
            Bb = 4 * g + i
            for c in range(2):
                lhsT = qTb[:, 2048 * c + 128 * Bb: 2048 * c + 128 * (Bb + 1)]
                mm(vscr[g][:, 128 * i:128 * (i + 1)], lhsT,
                   wv_sb[:, 128 * c:128 * (c + 1)],
                   start=(c == 0), stop=(c == 1), tile_position=(0, 0),
                   skip_group_check=True)
        if g % 2 == 0:
            nc.scalar.copy(Vr_sb[:, 512 * g:512 * (g + 1)], vscr[g][:])
        else:
            nc.vector.tensor_copy(Vr_sb[:, 512 * g:512 * (g + 1)], vscr[g][:])

    # ---- main loop ----
    # hidden pre-act y = a*dot + b*cost + c built fully in PSUM:
    #   dot: lhsT = K_bc4 (a-scaled K, s4-bcast), K=32, 4-row-packed
    #   affine: K=64 (cost rows + ones rows), 2+2 packed; adds b*cost + c
    # evacs are param-free -> merged 2-bank [128,1024] ops, scalar|vector.
    def emit_round(Bb, sc):
        slot = (Bb * NSC + sc) % 3
        hbase = 2048 * slot
        for wave in (0, 1):
            js = (0, 1) if wave == 0 else (2, 3)
            for j in js:
                lhsT = K_bc4[sc][32 * j:32 * j + 32, 128 * Bb:128 * (Bb + 1)]
                mm(hid_tile(j), lhsT, Q_sb[32 * j:32 * j + 32, :],
                   start=True, stop=False, tile_position=(32 * j, 0))
            for j in js:
                mm(hid_tile(j),
                   bpat_sb[32 * j:32 * j + 32, 128 * sc:128 * (sc + 1)],
                   cost_sb[32 * j:32 * j + 32, 512 * Bb:512 * (Bb + 1)],
                   start=False, stop=True, tile_position=(32 * j, 0))
            for j in js:
                t = sc * 4 + j
                dst = hid_sb[:, hbase + 512 * j: hbase + 512 * (j + 1)]
                if j % 2 == 0:
                    nc.scalar.activation(dst, hid_tile(j), AF.Relu,
                                         bias=cvec_sb[:, t:t + 1])
                else:
                    nc.vector.tensor_scalar(dst, hid_tile(j),
                                            cvec_sb[:, t:t + 1], None,
                                            op0=ALU.max)

    def emit_mix2(Bb, sc):
        slot = (Bb * NSC + sc) % 3
        hbase = 2048 * slot
        sps = sc_ps[Bb % 2]
        for j in range(4):
            mm(sps[32 * j:32 * j + 32, :],
               wpat_sb[:, 128 * sc + 32 * j: 128 * sc + 32 * (j + 1)],
               hid_sb[:, hbase + 512 * j: hbase + 512 * (j + 1)],
               start=(sc == 0), stop=(sc == NSC - 1), tile_position=(0, 32 * j),
               skip_group_check=True)

    def emit_exp(Bb):
        nc.scalar.activation(E_sb[:, 512 * (Bb % 3):512 * (Bb % 3 + 1)],
                             sc_ps[Bb % 2][:], AF.Exp)

    def emit_av(Bb):
        for j in range(4):
            mm(att_ps[32 * j:32 * j + 32, :],
               Vr_sb[32 * j:32 * j + 32, 128 * Bb + 32 * j:128 * Bb + 32 * (j + 1)],
               E_sb[32 * j:32 * j + 32, 512 * (Bb % 3):512 * (Bb % 3 + 1)],
               start=(Bb == 0), stop=(Bb == NB - 1), tile_position=(32 * j, 32 * j),
               skip_group_check=True)
        mm(sum_ps[0:4, :], spat_sb[:],
           E_sb[:, 512 * (Bb % 3):512 * (Bb % 3 + 1)],
           start=(Bb == 0), stop=(Bb == NB - 1), tile_position=(0, 0),
           skip_group_check=True)

    # software pipeline: mix2 lags rounds by one step; exp after mix2(sc=3);
    # AV lags exp by one B.
    steps = [(Bb, sc) for Bb in range(NB) for sc in range(NSC)]
    for idx, (Bb, sc) in enumerate(steps):
        emit_round(Bb, sc)
        if idx >= 1:
            pB, psc = steps[idx - 1]
            emit_mix2(pB, psc)
            if psc == NSC - 1:
                emit_exp(pB)
                if pB >= 1:
                    emit_av(pB - 1)
    emit_mix2(*steps[-1])
    emit_exp(NB - 1)
    emit_av(NB - 2)
    emit_av(NB - 1)

    # ---- tail: normalize + output projection ----
    nc.vector.reciprocal_approx_fast(zr_sb[0:4, :], sum_ps[0:4, :])
    mm(sc_ps[0][:], zpat_sb[0:4, 0:128], zr_sb[0:4, :],
       start=True, stop=True, tile_position=(0, 0))
    nc.scalar.copy(zb_sb[:], sc_ps[0][:])
    nc.vector.tensor_tensor(att_sb[:], att_ps[:], zb_sb[:], op=ALU.mult)
    for qc in range(4):
        ps = sc_ps[qc % 2]
        half = 256 * (qc // 2)
        mm(ps[:, half:half + 256], att_sb[:, 128 * qc:128 * (qc + 1)],
           wo_sb[:], start=True, stop=True, tile_position=(0, 0))
        eng = nc.vector.tensor_copy if qc % 2 else nc.scalar.copy
        eng(out_sb[:, 256 * qc:256 * (qc + 1)], ps[:, half:half + 256])
    dma(out_d.rearrange("(g p) d -> p g d", g=4),
        out_sb[:].rearrange("p (g d) -> p g d", g=4))
    ctx.close()


# --------------------------------------------------------------------------
# host-side input prep
# --------------------------------------------------------------------------
def make_core_inputs(inputs, core):
    b, quad = core // 2, core % 2
    queries = inputs["queries"][b]            # [512, 256]
    cost = inputs["cost_mat"][b]              # [512, 512]
    a = inputs["mix1_w"][:, 0, :]             # [H, MS]
    bb = inputs["mix1_w"][:, 1, :]
    cc = inputs["mix1_b"]                     # [H, MS]
    w2 = inputs["mix2_w"][:, :, 0]            # [H, MS]
    hs = slice(quad * 4 * DK, (quad + 1) * 4 * DK)

    qT = np.ascontiguousarray(queries.T).reshape(2, 128, 512)
    costT = np.ascontiguousarray(cost.T)      # [k, q]
    # cost replicas: rows 0-31 / 64-95 = cost block, rows 32-63 / 96-127 = ones
    # (the ones rows feed the affine matmul's c-bias contraction)
    costp = np.empty((NB, 128, 512), np.float32)
    for Bb in range(NB):
        blk = costT[32 * Bb:32 * Bb + 32, :]
        costp[Bb] = np.tile(blk, (4, 1))
    wk = np.ascontiguousarray(inputs["Wk"]).reshape(2, 128, 256)
    wq = (np.ascontiguousarray(inputs["Wq"]) * (DK ** -0.5)).astype(np.float32).reshape(2, 128, 256)
    # K/Q proj in the program use cols [256c : 256c+128] -> must be the quad's
    # 128 cols: bake quad slice so lhsT slice [*, :128] is the quad cols.
    wk = np.ascontiguousarray(wk[:, :, hs])   # [2,128,128]
    wq = np.ascontiguousarray(wq[:, :, hs])
    wk = np.concatenate([wk, np.zeros_like(wk)], axis=2)  # pad back to 256 cols
    wq = np.concatenate([wq, np.zeros_like(wq)], axis=2)
    wv = np.ascontiguousarray(inputs["Wv"][:, hs]).reshape(2, 128, 128)
    wo = np.ascontiguousarray(inputs["Wo"][hs, :])        # [128, 256]

    # bpat: b-diag at rows 32j (K=32, cost rhs); cvec: +c for ACT tiles
    # (j even, relu(x+c)), -c for DVE tiles (j odd, max(x,-c); the dropped
    # +c*w2 sum is k-uniform per head -> softmax-invariant).
    bpat = np.zeros((128, NSC * 128), np.float32)
    cvec = np.zeros((128, 16), np.float32)
    wpat = np.zeros((128, NSC * 128), np.float32)
    ak = np.zeros((128, 512), np.float32)
    rows = np.arange(32)
    for sc in range(NSC):
        for j in range(4):
            h = quad * 4 + j
            sgn = 1.0 if j % 2 == 0 else -1.0
            for si in range(4):
                s = sc * 4 + si
                bpat[32 * j + rows, 128 * sc + 32 * si + rows] = bb[h, s]
                cvec[32 * si + rows, sc * 4 + j] = sgn * cc[h, s]
                wpat[32 * si + rows, 128 * sc + 32 * j + rows] = w2[h, s]
    for p in range(128):
        h = quad * 4 + p // 32
        for s in range(MS):
            ak[p, 32 * s:32 * (s + 1)] = a[h, s]
    spat = np.zeros((128, 4), np.float32)
    for j in range(4):
        spat[32 * j:32 * (j + 1), j] = 1.0
    zpat = np.zeros((128, 128), np.float32)
    for j in range(4):
        zpat[j, 32 * j:32 * (j + 1)] = 1.0
    import ml_dtypes
    mmdt = ml_dtypes.bfloat16 if MM_FAST else np.float32
    # blob32 cols: zpat 128 | cvec 16
    blob32 = np.concatenate([zpat, cvec], axis=1).astype(np.float32)
    # blob16 cols: qT 1024 | wk 512 | wq 512 | wv 256 | wo 256 | bpat 512 |
    #              wpat 512 | spat 4 | ak 512
    blob16 = np.concatenate(
        [qT[0], qT[1], wk[0][:, :256], wk[1][:, :256],
         wq[0][:, :256], wq[1][:, :256], wv[0], wv[1], wo,
         bpat, wpat, spat, ak],
        axis=1).astype(mmdt)
    return dict(blob32=np.ascontiguousarray(blob32),
                blob16=np.ascontiguousarray(blob16),
                costp=costp.astype(mmdt))


def kernel(**inputs):
    global _last_results
    inputs = {k: np.asarray(v, np.float32) for k, v in inputs.items()}
    if "nc" not in _compiled:
        _compiled["nc"] = build_program()
    nc = _compiled["nc"]
    in_maps = [make_core_inputs(inputs, core) for core in range(8)]
    trace = bool(os.environ.get("MSK_TRACE"))
    if trace:
        _install_ntff_hook()
    res = run_bass_kernel_spmd(nc, in_maps, list(range(8)), trace=trace)
    _last_results = res
    out = np.zeros((B_, L, D), np.float32)
    for core in range(8):
        out[core // 2] += res.results[core]["out"]
    return out

